# revision 13
# baseline (speedup 1.0000x reference)
"""CLRNet SimOTA assignment kernel for Trainium2 (Bass/Tile).

Contract: kernel(**inputs) takes FULL inputs (preds [4,4096,78], targets
[4,32,78], masks [4,32], img_w, img_h) and returns (assigned [4,4096] bool,
matched [4,4096] int32) exactly like the reference.

Sharding: pure data parallel over the batch dim. B=4 images on 8 cores ->
each image runs on 2 cores (duplicated); outputs taken from cores 0..3.

Math notes (vs reference.py):
  - d[n,t,s] = |px[n,s] - tx[t,s]| masked by valid points.
    D[n,t] = sum_s d.  Computed as |PE-matmul diff| minus a correction:
    the matmul uses tx' = tx*valid (0 at invalid), so invalid points
    contribute |px| = px (px >= 0), removed by P[n,t] = px @ invalid_mask^T
    (a second matmul with the same stationary operand).
  - line IoU = (30*vcnt - D) / (30*vcnt + D + 1e-9): ovr elem = 30-|d|,
    union elem = 30+|d|.  Monotone decreasing in D per target, so the top-4
    iou priors per target are the 4 smallest-distance priors -> one Max8
    top-k (on negated distances) serves both iou top-k and dyn_k.
  - cost top-4 per target via Max8 on negated cost; selection is done by
    threshold (cost <= k-th smallest value), no indices needed.
  - No +/-inf anywhere: +/-1e30 sentinels (NaN-safe blending).
"""

import os
import sys

import numpy as np

for _p in ("/opt/trn_rl_repo", "/root/.axon_site/_ro/trn_rl_repo"):
    if os.path.isdir(_p) and _p not in sys.path:
        sys.path.insert(0, _p)

import concourse.bacc as bacc  # noqa: E402
import concourse.bass as bass  # noqa: E402
import concourse.mybir as mybir  # noqa: E402
from concourse import bass_isa  # noqa: E402
from concourse.tile import TileContext  # noqa: E402

F32 = mybir.dt.float32
I32 = mybir.dt.int32
U8 = mybir.dt.uint8
Alu = mybir.AluOpType
Act = mybir.ActivationFunctionType
AX = mybir.AxisListType

N = 4096
T = 32
S = 72
NT = N // 128  # 32 prior tiles of 128
TS = T * S  # 2304
BIG = 1.0e30
EPS = 1.0e-12

# matmul free-dim chunks of the [.., T*S] diff output, aligned to S-blocks
CHUNKS = [(0, 7), (7, 7), (14, 7), (21, 7), (28, 4)]  # (t_off, t_cnt)


def _host_constants():
    sel = np.zeros((S + 1, TS), np.float32)
    sel[:S, :] = np.tile(np.eye(S, dtype=np.float32), (1, T))
    ident = np.eye(128, dtype=np.float32)
    nident = -ident
    iotaq = np.tile(np.arange(8, dtype=np.float32)[None, :], (T, 1))
    iota32t = np.tile(np.arange(T, dtype=np.float32)[None, :], (128, 1))
    return sel, ident, nident, iotaq, iota32t


def build(img_w: float) -> bass.Bass:
    nc = bacc.Bacc("TRN2", target_bir_lowering=False, debug=False)

    preds_d = nc.dram_tensor("preds", [N, 78], F32, kind="ExternalInput")
    tgt_d = nc.dram_tensor("targets", [T, 78], F32, kind="ExternalInput")
    mask_d = nc.dram_tensor("masks", [1, T], I32, kind="ExternalInput")
    sel_d = nc.dram_tensor("sel73", [S + 1, TS], F32, kind="ExternalInput")
    id_d = nc.dram_tensor("ident", [128, 128], F32, kind="ExternalInput")
    nid_d = nc.dram_tensor("nident", [128, 128], F32, kind="ExternalInput")
    iq_d = nc.dram_tensor("iotaq", [T, 8], F32, kind="ExternalInput")
    it_d = nc.dram_tensor("iota32t", [128, T], F32, kind="ExternalInput")
    asn_d = nc.dram_tensor("assigned", [N], U8, kind="ExternalOutput")
    mat_d = nc.dram_tensor("matched", [N], I32, kind="ExternalOutput")

    with TileContext(nc) as tc:
        with (
            tc.tile_pool(name="const", bufs=1) as constp,
            tc.tile_pool(name="setup", bufs=1) as setp,
            tc.tile_pool(name="big", bufs=1) as bigp,
            tc.tile_pool(name="ptile", bufs=3) as predp,
            tc.tile_pool(name="stile", bufs=2) as sp,
            tc.tile_pool(name="work", bufs=3) as wp,
            tc.tile_pool(name="small", bufs=2) as smp,
            tc.tile_pool(name="psT", bufs=2, space="PSUM") as psT,
            tc.tile_pool(name="psD", bufs=2, space="PSUM") as psD,
            tc.tile_pool(name="psM", bufs=2, space="PSUM") as psM,
        ):
            v = nc.vector
            sc = nc.scalar
            gp = nc.gpsimd
            te = nc.tensor

            # ---------------- constants / inputs ----------------
            selrhs = bigp.tile([S + 1, TS], F32)
            nc.sync.dma_start(out=selrhs[:], in_=sel_d.ap())
            ident = constp.tile([128, 128], F32)
            nc.sync.dma_start(out=ident[:], in_=id_d.ap())
            nident = constp.tile([128, 128], F32)
            nc.sync.dma_start(out=nident[:], in_=nid_d.ap())
            iotaq = constp.tile([T, 8], F32)
            nc.sync.dma_start(out=iotaq[:], in_=iq_d.ap())
            iota32t = constp.tile([128, T], F32)
            nc.sync.dma_start(out=iota32t[:], in_=it_d.ap())

            tgt = setp.tile([T, 78], F32)
            nc.sync.dma_start(out=tgt[:], in_=tgt_d.ap())
            maskrow_i = setp.tile([1, T], I32)
            nc.sync.dma_start(out=maskrow_i[:], in_=mask_d.ap())
            # same dram tensor, column layout [T,1]
            maskcol_i = setp.tile([T, 1], I32)
            nc.sync.dma_start(
                out=maskcol_i[:], in_=bass.AP(mask_d, 0, [[1, T], [1, 1]])
            )
            maskrow = setp.tile([1, T], F32)
            v.tensor_copy(maskrow[:], maskrow_i[:])
            maskcol = setp.tile([T, 1], F32)
            v.tensor_copy(maskcol[:], maskcol_i[:])

            # ---------------- target-side prep (t-major [T, ...]) -------
            tx = tgt[:, 6:78]  # [T,S]
            mge = setp.tile([T, S], F32)
            v.tensor_scalar(mge[:], tx, 0.0, None, op0=Alu.is_ge)
            mv = setp.tile([T, S], F32)  # valid-point mask
            v.tensor_scalar(mv[:], tx, float(img_w), None, op0=Alu.is_lt)
            v.tensor_tensor(out=mv[:], in0=mv[:], in1=mge[:], op=Alu.mult)

            vcnt = setp.tile([T, 1], F32)
            v.tensor_reduce(vcnt[:], mv[:], axis=AX.X, op=Alu.add)
            thirty = setp.tile([T, 1], F32)
            v.tensor_scalar(thirty[:], vcnt[:], 30.0, None, op0=Alu.mult)
            thirty_eps = setp.tile([T, 1], F32)
            v.tensor_scalar(
                thirty_eps[:], vcnt[:], 30.0, 1.0e-9, op0=Alu.mult, op1=Alu.add
            )
            vlenp = setp.tile([T, 1], F32)
            v.tensor_scalar(
                vlenp[:], vcnt[:], 1.0, 1.0e-6, op0=Alu.max, op1=Alu.add
            )
            invvlen = setp.tile([T, 1], F32)
            v.reciprocal(invvlen[:], vlenp[:])
            nvlenp = setp.tile([T, 1], F32)
            v.tensor_scalar(nvlenp[:], vlenp[:], -1.0, None, op0=Alu.mult)

            # tx' = tx * mv, negated -> row 72 of the selector rhs
            mneg = setp.tile([T, S], F32)
            v.tensor_scalar(mneg[:], mv[:], -1.0, None, op0=Alu.mult)
            txn = setp.tile([T, S], F32)
            v.tensor_tensor(out=txn[:], in0=tx, in1=mneg[:], op=Alu.mult)
            nc.sync.dma_start(out=selrhs[S : S + 1, :], in_=txn[:])

            # invalid mask, transposed -> rhs_small [S+1, T]
            invm = setp.tile([T, S], F32)
            v.tensor_scalar(
                invm[:], mv[:], -1.0, 1.0, op0=Alu.mult, op1=Alu.add
            )
            p_invT = psT.tile([S, T], F32, tag="tr")
            te.transpose(p_invT[:], invm[:], ident[0:T, 0:T])
            rhs_small = setp.tile([S + 1, T], F32)
            v.memset(rhs_small[:], 0.0)
            sc.copy(rhs_small[0:S, :], p_invT[:])

            # target feature rows straight from DRAM ([1, T] strided loads)
            def tgt_row(col, name):
                t = setp.tile([1, T], F32, tag=name + "r")
                nc.sync.dma_start(
                    out=t[:], in_=bass.AP(tgt_d, col, [[1, 1], [78, T]])
                )
                return t

            def bcast128(src_row, name):
                t = setp.tile([128, T], F32, tag=name)
                gp.partition_broadcast(t[:], src_row)
                return t

            labb = bcast128(tgt_row(1, "labb")[:], "labb")
            sxtb = bcast128(tgt_row(2, "sxtb")[:], "sxtb")
            sytb = bcast128(tgt_row(3, "sytb")[:], "sytb")
            thtb = bcast128(tgt_row(4, "thtb")[:], "thtb")
            validb = bcast128(maskrow[:], "validb")
            # +BIG where valid, -BIG where invalid (tt-min blend mask)
            maskminb = setp.tile([128, T], F32)
            v.tensor_scalar(
                maskminb[:], validb[:], 2.0 * BIG, -BIG,
                op0=Alu.mult, op1=Alu.add,
            )
            # invvlen broadcast [128, T]
            invr = setp.tile([1, T], F32)
            nc.sync.dma_start(out=invr[:], in_=invvlen[:])
            invvlenb = bcast128(invr[:], "invvlenb")

            # ---------------- focal-loss E columns ----------------
            # E[:, 2i+c] = pos - neg for prior-tile i, class c
            eln = setp.tile([128, 2 * NT], F32)
            nc.sync.dma_start(
                out=eln[:],
                in_=bass.AP(preds_d, 0, [[78, 128], [78 * 128, NT], [1, 2]]),
            )
            ceps = setp.tile([128, 1], F32)
            v.memset(ceps[:], EPS)
            c1peps = setp.tile([128, 1], F32)
            v.memset(c1peps[:], 1.0 + EPS)
            pr = setp.tile([128, 2 * NT], F32)
            sc.activation(pr[:], eln[:], Act.Sigmoid)
            l1 = setp.tile([128, 2 * NT], F32)
            sc.activation(l1[:], pr[:], Act.Ln, bias=ceps[:], scale=1.0)
            l2 = setp.tile([128, 2 * NT], F32)
            sc.activation(l2[:], pr[:], Act.Ln, bias=c1peps[:], scale=-1.0)
            q2 = setp.tile([128, 2 * NT], F32)
            sc.activation(q2[:], pr[:], Act.Square, bias=1.0, scale=-1.0)
            p2 = setp.tile([128, 2 * NT], F32)
            sc.activation(p2[:], pr[:], Act.Square)
            m1e = setp.tile([128, 2 * NT], F32)
            v.tensor_tensor(out=m1e[:], in0=l1[:], in1=q2[:], op=Alu.mult)
            m2e = setp.tile([128, 2 * NT], F32)
            v.tensor_tensor(out=m2e[:], in0=l2[:], in1=p2[:], op=Alu.mult)
            v.tensor_scalar(m2e[:], m2e[:], 0.75, None, op0=Alu.mult)
            ecols = setp.tile([128, 2 * NT], F32)
            v.scalar_tensor_tensor(
                ecols[:], m1e[:], -0.25, m2e[:], op0=Alu.mult, op1=Alu.add
            )
            de_all = setp.tile([128, NT], F32)
            v.tensor_tensor(
                out=de_all[:],
                in0=bass.AP(ecols.tensor, 1, [[2 * NT, 128], [2, NT]]),
                in1=bass.AP(ecols.tensor, 0, [[2 * NT, 128], [2, NT]]),
                op=Alu.subtract,
            )

            # batched per-prior matrices [128, NT*T] (columns = (tile, t))
            dist_all = bigp.tile([128, NT * T], F32)
            sd_all = bigp.tile([128, NT * T], F32)
            td_all = bigp.tile([128, NT * T], F32)
            cls_all = bigp.tile([128, NT * T], F32)

            # ---------------- phase 1: per prior-tile ----------------
            for i in range(NT):
                pt = predp.tile([128, 78], F32, tag="pt")
                nc.sync.dma_start(
                    out=pt[:], in_=preds_d.ap()[i * 128 : (i + 1) * 128, :]
                )
                # stationary S = [px^T ; ones]
                p_pT = psT.tile([S, 128], F32, tag="tr")
                te.transpose(p_pT[:], pt[:, 6:78], ident[:])
                smat = sp.tile([S + 1, 128], F32, tag="smat")
                v.memset(smat[:], 1.0)
                sc.copy(smat[0:S, :], p_pT[:])

                draw = wp.tile([128, T], F32, tag="draw")
                for (toff, tcnt) in CHUNKS:
                    fsz = tcnt * S
                    pdiff = psD.tile([128, 504], F32, tag="pdiff")
                    te.matmul(
                        pdiff[:, 0:fsz],
                        smat[:],
                        selrhs[:, toff * S : toff * S + fsz],
                        start=True,
                        stop=True,
                    )
                    adc = wp.tile([128, 504], F32, tag="adc")
                    sc.activation(adc[:, 0:fsz], pdiff[:, 0:fsz], Act.Abs)
                    v.tensor_reduce(
                        draw[:, toff : toff + tcnt],
                        adc[:, 0:fsz].rearrange("p (t s) -> p t s", s=S),
                        axis=AX.X,
                        op=Alu.add,
                    )
                # invalid-point correction
                pp = psM.tile([128, T], F32, tag="m")
                te.matmul(pp[:], smat[:], rhs_small[:], start=True, stop=True)
                dcol = dist_all[:, i * T : (i + 1) * T]
                v.tensor_tensor(out=dcol, in0=draw[:], in1=pp[:], op=Alu.subtract)
                v.tensor_tensor(out=dcol, in0=dcol, in1=invvlenb[:], op=Alu.mult)

                # start-point distance (exact path)
                dx = wp.tile([128, T], F32, tag="dx")
                v.tensor_scalar(dx[:], sxtb[:], pt[:, 2:3], None, op0=Alu.subtract)
                dy = wp.tile([128, T], F32, tag="dy")
                v.tensor_scalar(dy[:], sytb[:], pt[:, 3:4], None, op0=Alu.subtract)
                sc.activation(dx[:], dx[:], Act.Square)
                sc.activation(dy[:], dy[:], Act.Square)
                v.tensor_tensor(out=dx[:], in0=dx[:], in1=dy[:], op=Alu.add)
                scol = sd_all[:, i * T : (i + 1) * T]
                sc.activation(scol, dx[:], Act.Sqrt)

                # theta distance
                tdr = wp.tile([128, T], F32, tag="tdr")
                v.tensor_scalar(tdr[:], thtb[:], pt[:, 4:5], None, op0=Alu.subtract)
                tcol = td_all[:, i * T : (i + 1) * T]
                sc.activation(tcol, tdr[:], Act.Abs)

                # focal classification cost column
                ccol = cls_all[:, i * T : (i + 1) * T]
                v.tensor_scalar(
                    ccol, labb[:], de_all[:, i : i + 1], ecols[:, 2 * i : 2 * i + 1],
                    op0=Alu.mult, op1=Alu.add,
                )

            # ---------------- stats -> -1/max scalars ----------------
            # masked max over (all priors, valid targets) of a [128, NT*T]
            # matrix, computed post-hoc on DVE (gpsimd has no tensor ops)
            mk3s = maskminb[:].unsqueeze(1).broadcast_to([128, NT, T])

            def neg_inv_max(acc, name):
                mm = smp.tile([128, NT * T], F32, tag="statscratch")
                v.tensor_tensor(
                    out=mm[:].rearrange("p (i t) -> p i t", t=T),
                    in0=acc[:].rearrange("p (i t) -> p i t", t=T),
                    in1=mk3s,
                    op=Alu.min,
                )
                r = smp.tile([128, 1], F32, tag=name + "r")
                v.tensor_reduce(r[:], mm[:], axis=AX.X, op=Alu.max)
                ar = smp.tile([128, 1], F32, tag=name + "ar")
                gp.partition_all_reduce(
                    ar[:], r[:], channels=128, reduce_op=bass_isa.ReduceOp.max
                )
                v.tensor_scalar(ar[:], ar[:], 1.0e-6, None, op0=Alu.max)
                inv = smp.tile([128, 1], F32, tag=name + "inv")
                v.reciprocal(inv[:], ar[:])
                v.tensor_scalar(inv[:], inv[:], -1.0, None, op0=Alu.mult)
                return inv

            ninvd = neg_inv_max(dist_all, "d")
            ninvs = neg_inv_max(sd_all, "s")
            ninvt = neg_inv_max(td_all, "t")

            # ---------------- phase 2: cost (batched) ----------------
            NTT = NT * T
            a_ = bigp.tile([128, NTT], F32)
            v.tensor_scalar(a_[:], dist_all[:], ninvd[:], 1.01, op0=Alu.mult, op1=Alu.add)
            b_ = bigp.tile([128, NTT], F32)
            v.tensor_scalar(b_[:], sd_all[:], ninvs[:], 1.01, op0=Alu.mult, op1=Alu.add)
            c_ = bigp.tile([128, NTT], F32)
            v.tensor_scalar(c_[:], td_all[:], ninvt[:], 1.01, op0=Alu.mult, op1=Alu.add)
            v.tensor_tensor(out=a_[:], in0=a_[:], in1=b_[:], op=Alu.mult)
            v.tensor_tensor(out=a_[:], in0=a_[:], in1=c_[:], op=Alu.mult)
            # clamp: only reachable for invalid-target columns (valid ones
            # have scores in [0.01, 1.01]); keeps square/negcost finite
            v.tensor_scalar(
                a_[:], a_[:], -1.0e17, 1.0e17, op0=Alu.max, op1=Alu.min
            )
            sq = b_  # reuse
            sc.activation(sq[:], a_[:], Act.Square)
            negcost = bigp.tile([128, NTT], F32)
            v.scalar_tensor_tensor(
                negcost[:], sq[:], 3.0, cls_all[:], op0=Alu.mult, op1=Alu.subtract
            )
            mk3 = maskminb[:].unsqueeze(1).broadcast_to([128, NT, T])
            nv3 = negcost[:].rearrange("p (i t) -> p i t", t=T)
            v.tensor_tensor(out=nv3, in0=nv3, in1=mk3, op=Alu.min)

            # ---------------- fold + Max8 top-4 ----------------
            def col_top8(mat_all, neg_identity):
                """Per-target top-8 of (sign * columns) over all N priors.

                mat_all: [128, NT*T]; returns [T, 8] descending of the
                negated (if neg_identity) column values.
                """
                cands = smp.tile([T, 64], F32, tag="cands")
                for g in range(8):
                    psg = psM.tile([T, 512], F32, tag="m")
                    for j in range(4):
                        i = 4 * g + j
                        te.matmul(
                            psg[:, j * 128 : (j + 1) * 128],
                            mat_all[:, i * T : (i + 1) * T],
                            neg_identity[:],
                            start=True,
                            stop=True,
                        )
                    v.max(out=cands[:, 8 * g : 8 * (g + 1)], in_=psg[:])
                top8 = smp.tile([T, 8], F32, tag="top8")
                v.max(out=top8[:], in_=cands[:])
                return top8

            nd8 = col_top8(dist_all, nident)  # -distances, descending
            # iou of the 4 closest priors per target
            dq = smp.tile([T, 4], F32, tag="dq")
            v.tensor_scalar(dq[:], nd8[:, 0:4], nvlenp[:], None, op0=Alu.mult)
            numq = smp.tile([T, 4], F32, tag="numq")
            v.tensor_scalar(numq[:], dq[:], -1.0, thirty[:], op0=Alu.mult, op1=Alu.add)
            denq = smp.tile([T, 4], F32, tag="denq")
            v.tensor_scalar(denq[:], dq[:], thirty_eps[:], None, op0=Alu.add)
            v.reciprocal(denq[:], denq[:])
            v.tensor_tensor(out=numq[:], in0=numq[:], in1=denq[:], op=Alu.mult)
            v.tensor_scalar(numq[:], numq[:], 0.0, None, op0=Alu.max)
            s4 = smp.tile([T, 1], F32, tag="s4")
            v.tensor_reduce(s4[:], numq[:], axis=AX.X, op=Alu.add)
            # km1 = clip(int(s4),1,4) - 1 = is_ge(s4,2)+is_ge(s4,3)+is_ge(s4,4)
            km1 = smp.tile([T, 1], F32, tag="km1")
            g3 = smp.tile([T, 1], F32, tag="g3")
            v.tensor_scalar(km1[:], s4[:], 2.0, None, op0=Alu.is_ge)
            v.tensor_scalar(g3[:], s4[:], 3.0, None, op0=Alu.is_ge)
            v.tensor_tensor(out=km1[:], in0=km1[:], in1=g3[:], op=Alu.add)
            v.tensor_scalar(g3[:], s4[:], 4.0, None, op0=Alu.is_ge)
            v.tensor_tensor(out=km1[:], in0=km1[:], in1=g3[:], op=Alu.add)

            nc8 = col_top8(negcost, ident)  # -cost ascending-cost, descending
            eqk = smp.tile([T, 4], F32, tag="eqk")
            v.tensor_scalar(eqk[:], iotaq[:, 0:4], km1[:], None, op0=Alu.is_equal)
            v.tensor_tensor(out=eqk[:], in0=eqk[:], in1=nc8[:, 0:4], op=Alu.mult)
            ck = smp.tile([T, 1], F32, tag="ck")
            v.tensor_reduce(ck[:], eqk[:], axis=AX.X, op=Alu.add)
            # invalid targets -> +BIG threshold (select nothing)
            v.tensor_tensor(out=ck[:], in0=ck[:], in1=maskcol[:], op=Alu.mult)
            mb = smp.tile([T, 1], F32, tag="mb")
            v.tensor_scalar(mb[:], maskcol[:], -BIG, BIG, op0=Alu.mult, op1=Alu.add)
            v.tensor_tensor(out=ck[:], in0=ck[:], in1=mb[:], op=Alu.add)
            ckr = smp.tile([1, T], F32, tag="ckr")
            nc.sync.dma_start(out=ckr[:], in_=ck[:])
            ckb = smp.tile([128, T], F32, tag="ckb")
            gp.partition_broadcast(ckb[:], ckr[:])

            # ---------------- selection / conflict / outputs ----------
            mm = bigp.tile([128, NTT], F32)
            ck3 = ckb[:].unsqueeze(1).broadcast_to([128, NT, T])
            mm3 = mm[:].rearrange("p (i t) -> p i t", t=T)
            v.tensor_tensor(out=mm3, in0=nv3, in1=ck3, op=Alu.is_ge)
            rs = smp.tile([128, NT], F32, tag="rs")
            v.tensor_reduce(rs[:], mm3, axis=AX.X, op=Alu.add)
            nmax = smp.tile([128, NT], F32, tag="nmax")
            v.tensor_reduce(nmax[:], nv3, axis=AX.X, op=Alu.max)
            oh = bigp.tile([128, NTT], F32)
            oh3 = oh[:].rearrange("p (i t) -> p i t", t=T)
            nm3 = nmax[:].unsqueeze(2).broadcast_to([128, NT, T])
            v.tensor_tensor(out=oh3, in0=nv3, in1=nm3, op=Alu.is_equal)
            conf = smp.tile([128, NT], F32, tag="conf")
            v.tensor_scalar(conf[:], rs[:], 1.0, None, op0=Alu.is_gt)
            cf3 = conf[:].unsqueeze(2).broadcast_to([128, NT, T])
            # mm = mm + conf * (oh - mm)   (conflict rows -> one-hot argmin)
            v.tensor_tensor(out=oh3, in0=oh3, in1=mm3, op=Alu.subtract)
            v.tensor_tensor(out=oh3, in0=oh3, in1=cf3, op=Alu.mult)
            v.tensor_tensor(out=mm[:], in0=mm[:], in1=oh[:], op=Alu.add)
            asum = smp.tile([128, NT], F32, tag="asum")
            v.tensor_reduce(asum[:], mm3, axis=AX.X, op=Alu.max)
            t1 = bigp.tile([128, NTT], F32)
            v.tensor_scalar(t1[:], mm[:], -1.0e9, 1.0e9, op0=Alu.mult, op1=Alu.add)
            it3 = iota32t[:].unsqueeze(1).broadcast_to([128, NT, T])
            t13 = t1[:].rearrange("p (i t) -> p i t", t=T)
            v.tensor_tensor(out=t13, in0=t13, in1=it3, op=Alu.add)
            idxm = smp.tile([128, NT], F32, tag="idxm")
            v.tensor_reduce(idxm[:], t13, axis=AX.X, op=Alu.min)
            v.tensor_scalar(idxm[:], idxm[:], 1.0, None, op0=Alu.add)
            v.tensor_tensor(out=idxm[:], in0=idxm[:], in1=asum[:], op=Alu.mult)
            v.tensor_scalar(idxm[:], idxm[:], -1.0, None, op0=Alu.add)

            mi32 = smp.tile([128, NT], I32, tag="mi32")
            v.tensor_copy(mi32[:], idxm[:])
            au8 = smp.tile([128, NT], U8, tag="au8")
            v.tensor_copy(au8[:], asum[:])
            nc.sync.dma_start(
                out=bass.AP(mat_d, 0, [[1, 128], [128, NT]]), in_=mi32[:]
            )
            nc.sync.dma_start(
                out=bass.AP(asn_d, 0, [[1, 128], [128, NT]]), in_=au8[:]
            )

    nc.compile()
    return nc


_CACHE: dict[float, bass.Bass] = {}


def _get_nc(img_w: float) -> bass.Bass:
    if img_w not in _CACHE:
        _CACHE[img_w] = build(img_w)
    return _CACHE[img_w]


def kernel(preds, targets, masks, img_w, img_h):
    del img_h
    B = preds.shape[0]
    n_cores = 8
    nc = _get_nc(float(img_w))
    sel, ident, nident, iotaq, iota32t = _host_constants()
    in_maps = []
    for c in range(n_cores):
        b = c % B
        in_maps.append(
            {
                "preds": np.ascontiguousarray(preds[b], np.float32),
                "targets": np.ascontiguousarray(targets[b], np.float32),
                "masks": np.ascontiguousarray(
                    masks[b].reshape(1, T), np.int32
                ),
                "sel73": sel,
                "ident": ident,
                "nident": nident,
                "iotaq": iotaq,
                "iota32t": iota32t,
            }
        )
    from concourse.bass_utils import run_bass_kernel_spmd

    res = run_bass_kernel_spmd(nc, in_maps, list(range(n_cores)))
    outs = res.results[:B]
    assigned = np.stack([o["assigned"] for o in outs]).astype(bool)
    matched = np.stack([o["matched"] for o in outs]).astype(np.int32)
    return assigned, matched


# revision 17
# speedup vs baseline: 1.9361x; 1.9361x over previous
"""CLRNet SimOTA assignment kernel for Trainium2 (Bass/Tile).

Contract: kernel(**inputs) takes FULL inputs (preds [4,4096,78], targets
[4,32,78], masks [4,32], img_w, img_h) and returns (assigned [4,4096] bool,
matched [4,4096] int32) exactly like the reference.

Sharding: pure data parallel over the batch dim. B=4 images on 8 cores ->
each image runs on 2 cores (duplicated); outputs taken from cores 0..3.

Key design (driven by measured TRN2 instruction costs):
  - D[n,t] = sum_s valid*|px[n,s]-tx[t,s]| via ONE custom-DVE scan op per
    prior tile: running prefix sum of |px_bcast - tx'| over the (t,s) free
    dim; per-target sums recovered as differences of block-end prefixes.
    tx' = tx*valid makes invalid points contribute |px| = px, removed by
    P[n,t] = px @ invalid_mask^T computed on the (otherwise idle) PE.
  - line IoU = (30*vcnt - D)/(30*vcnt + D + 1e-9) (ovr elem = 30-|d|,
    union elem = 30+|d|), monotone decreasing in D per target, so top-4
    iou priors = 4 smallest-distance priors -> Max8 on negated distances.
  - cost top-4 per target via Max8 on negated cost; selection by value
    threshold (cost <= k-th smallest), no indices anywhere.
  - All per-tile small ops are batched into [128, NT*T] passes using
    step-0 broadcast access patterns.
  - No +/-inf: +/-1e30 sentinels.
"""

import os
import sys

import numpy as np

for _p in ("/opt/trn_rl_repo", "/root/.axon_site/_ro/trn_rl_repo"):
    if os.path.isdir(_p) and _p not in sys.path:
        sys.path.insert(0, _p)

import concourse.bacc as bacc  # noqa: E402
import concourse.bass as bass  # noqa: E402
import concourse.mybir as mybir  # noqa: E402
from concourse import bass_isa, dve_ops  # noqa: E402
from concourse.dve_spec import AluOp as DAlu  # noqa: E402
from concourse.dve_spec import Bin, Spec, Src0, Src1, lower, scan  # noqa: E402
from concourse.dve_uop import DveOpSpec  # noqa: E402
from concourse.tile import TileContext  # noqa: E402

F32 = mybir.dt.float32
I32 = mybir.dt.int32
U8 = mybir.dt.uint8
Alu = mybir.AluOpType
Act = mybir.ActivationFunctionType
AX = mybir.AxisListType

N = 4096
T = 32
S = 72
NT = N // 128  # 32 prior tiles of 128
NTT = NT * T
TS = T * S  # 2304
BIG = 1.0e30
EPS = 1.0e-12


def _register_absdiff_scan():
    """Custom DVE op: out[p,k] = prefix-sum of |in0 - in1| along free dim."""
    name = "ABSDIFF_SCAN_CLR"
    for op in dve_ops.OPS:
        if op.name == name:
            return op

    def _ref(in0, in1, s0, s1, imm2):
        a = in0.astype(np.float32).reshape(in0.shape[0], -1)
        b = in1.astype(np.float32).reshape(in1.shape[0], -1)
        d = np.abs(a - b)
        return np.cumsum(d, axis=-1).astype(np.float32)

    spec = Spec(
        body=scan(DAlu.ADD, Bin(DAlu.ABSOLUTE_DIFF, Src0, Src1)),
        reference=_ref,
    )
    shas = {}
    for ver in ("v3", "v4"):
        u = lower(spec, ver=ver)
        shas[ver] = DveOpSpec(name=name, opcode=0, uops=u, rd1_en=True).sha(ver)
    op = dve_ops.DveOp(name, spec, subdim=False, uops_sha=shas)
    dve_ops.OPS.append(op)
    dve_ops.CUSTOM_DVE_SPECS[name] = spec
    dve_ops._SUB_OPCODE_FOR_NAME[name] = (
        dve_ops._CUSTOM_DVE_ROW_BASE + len(dve_ops.OPS) - 1
    )
    return op


ABSDIFF_SCAN = _register_absdiff_scan()


def _host_constants():
    ident = np.eye(128, dtype=np.float32)
    iotaq = np.tile(np.arange(8, dtype=np.float32)[None, :], (T, 1))
    iota32t = np.tile(np.arange(T, dtype=np.float32)[None, :], (128, 1))
    return ident, iotaq, iota32t


def _b3(ap2d, axis):
    """[128, T]/[128, NT] -> broadcast [128, NT, T] view (step-0 dim)."""
    if axis == "tile":  # per-t data, broadcast over tile dim
        return ap2d.unsqueeze(1).broadcast_to([128, NT, T])
    return ap2d.unsqueeze(2).broadcast_to([128, NT, T])  # per-tile data


def build(img_w: float) -> bass.Bass:
    nc = bacc.Bacc("TRN2", target_bir_lowering=False, debug=False)

    preds_d = nc.dram_tensor("preds", [N, 78], F32, kind="ExternalInput")
    tgt_d = nc.dram_tensor("targets", [T, 78], F32, kind="ExternalInput")
    mask_d = nc.dram_tensor("masks", [1, T], I32, kind="ExternalInput")
    id_d = nc.dram_tensor("ident", [128, 128], F32, kind="ExternalInput")
    iq_d = nc.dram_tensor("iotaq", [T, 8], F32, kind="ExternalInput")
    it_d = nc.dram_tensor("iota32t", [128, T], F32, kind="ExternalInput")
    asn_d = nc.dram_tensor("assigned", [N], U8, kind="ExternalOutput")
    mat_d = nc.dram_tensor("matched", [N], I32, kind="ExternalOutput")

    with TileContext(nc) as tc:
        with (
            tc.tile_pool(name="const", bufs=1) as constp,
            tc.tile_pool(name="setup", bufs=1) as setp,
            tc.tile_pool(name="big", bufs=1) as bigp,
            tc.tile_pool(name="ptile", bufs=3) as predp,
            tc.tile_pool(name="pfxp", bufs=2) as pfxp,
            tc.tile_pool(name="stile", bufs=2) as sp,
            tc.tile_pool(name="small", bufs=2) as smp,
            tc.tile_pool(name="psT", bufs=2, space="PSUM") as psT,
            tc.tile_pool(name="psP", bufs=2, space="PSUM") as psP,
            tc.tile_pool(name="psF", bufs=2, space="PSUM") as psF,
        ):
            v = nc.vector
            sc = nc.scalar
            gp = nc.gpsimd
            te = nc.tensor

            # ---------------- constants / inputs ----------------
            ident = constp.tile([128, 128], F32)
            nc.sync.dma_start(out=ident[:], in_=id_d.ap())
            iotaq = constp.tile([T, 8], F32)
            nc.sync.dma_start(out=iotaq[:], in_=iq_d.ap())
            iota32t = constp.tile([128, T], F32)
            nc.sync.dma_start(out=iota32t[:], in_=it_d.ap())

            tgt = setp.tile([T, 78], F32)
            nc.sync.dma_start(out=tgt[:], in_=tgt_d.ap())
            maskrow_i = setp.tile([1, T], I32)
            nc.sync.dma_start(out=maskrow_i[:], in_=mask_d.ap())
            maskcol_i = setp.tile([T, 1], I32)
            nc.sync.dma_start(
                out=maskcol_i[:], in_=bass.AP(mask_d, 0, [[1, T], [1, 1]])
            )
            maskrow = setp.tile([1, T], F32)
            v.tensor_copy(maskrow[:], maskrow_i[:])
            maskcol = setp.tile([T, 1], F32)
            v.tensor_copy(maskcol[:], maskcol_i[:])

            # per-(prior-tile) feature columns: sx, sy, theta  [128, NT]
            def feat_cols(col, name):
                t = setp.tile([128, NT], F32, tag=name)
                nc.sync.dma_start(
                    out=t[:],
                    in_=bass.AP(preds_d, col, [[78, 128], [78 * 128, NT], [1, 1]]),
                )
                return t

            sxp = feat_cols(2, "sxp")
            syp = feat_cols(3, "syp")
            thp = feat_cols(4, "thp")

            # ---------------- target-side prep (t-major [T, ...]) -------
            tx = tgt[:, 6:78]  # [T,S]
            mge = setp.tile([T, S], F32)
            v.tensor_scalar(mge[:], tx, 0.0, None, op0=Alu.is_ge)
            mv = setp.tile([T, S], F32)  # valid-point mask
            v.tensor_scalar(mv[:], tx, float(img_w), None, op0=Alu.is_lt)
            v.tensor_tensor(out=mv[:], in0=mv[:], in1=mge[:], op=Alu.mult)

            vcnt = setp.tile([T, 1], F32)
            v.tensor_reduce(vcnt[:], mv[:], axis=AX.X, op=Alu.add)
            thirty = setp.tile([T, 1], F32)
            v.tensor_scalar(thirty[:], vcnt[:], 30.0, None, op0=Alu.mult)
            thirty_eps = setp.tile([T, 1], F32)
            v.tensor_scalar(
                thirty_eps[:], vcnt[:], 30.0, 1.0e-9, op0=Alu.mult, op1=Alu.add
            )
            vlenp = setp.tile([T, 1], F32)
            v.tensor_scalar(vlenp[:], vcnt[:], 1.0, 1.0e-6, op0=Alu.max, op1=Alu.add)
            invvlen = setp.tile([T, 1], F32)
            v.reciprocal(invvlen[:], vlenp[:])
            nvlenp = setp.tile([T, 1], F32)
            v.tensor_scalar(nvlenp[:], vlenp[:], -1.0, None, op0=Alu.mult)

            # tx' = tx * mv -> flat row -> broadcast [128, TS]
            txn = setp.tile([T, S], F32)
            v.tensor_tensor(out=txn[:], in0=tx, in1=mv[:], op=Alu.mult)
            txrow = setp.tile([1, TS], F32)
            nc.sync.dma_start(out=txrow[:], in_=txn[:])
            txb = bigp.tile([128, TS], F32)
            gp.partition_broadcast(txb[:], txrow[:])

            # invalid-mask transposed -> P matmul rhs [S, T]
            invm = setp.tile([T, S], F32)
            v.tensor_scalar(invm[:], mv[:], -1.0, 1.0, op0=Alu.mult, op1=Alu.add)
            p_invT = psT.tile([S, T], F32, tag="tr")
            te.transpose(p_invT[:], invm[:], ident[0:T, 0:T])
            invmC = setp.tile([S, T], F32)
            sc.copy(invmC[:], p_invT[:])

            # target feature rows -> [128, T] broadcasts
            def tgt_row(col, name):
                t = setp.tile([1, T], F32, tag=name + "r")
                nc.sync.dma_start(
                    out=t[:], in_=bass.AP(tgt_d, col, [[1, 1], [78, T]])
                )
                return t

            def bcast128(src_row, name):
                t = setp.tile([128, T], F32, tag=name)
                gp.partition_broadcast(t[:], src_row)
                return t

            labb = bcast128(tgt_row(1, "labb")[:], "labb")
            sxtb = bcast128(tgt_row(2, "sxtb")[:], "sxtb")
            sytb = bcast128(tgt_row(3, "sytb")[:], "sytb")
            thtb = bcast128(tgt_row(4, "thtb")[:], "thtb")
            validb = bcast128(maskrow[:], "validb")
            maskminb = setp.tile([128, T], F32)  # +BIG valid / -BIG invalid
            v.tensor_scalar(
                maskminb[:], validb[:], 2.0 * BIG, -BIG, op0=Alu.mult, op1=Alu.add
            )
            bigmask = setp.tile([128, T], F32)  # 0 valid / +BIG invalid
            v.tensor_scalar(
                bigmask[:], validb[:], -BIG, BIG, op0=Alu.mult, op1=Alu.add
            )
            invr = setp.tile([1, T], F32)
            nc.sync.dma_start(out=invr[:], in_=invvlen[:])
            invvlenb = bcast128(invr[:], "invvlenb")

            # ---------------- focal-loss E columns ----------------
            ceps = setp.tile([128, 1], F32)
            v.memset(ceps[:], EPS)
            c1peps = setp.tile([128, 1], F32)
            v.memset(c1peps[:], 1.0 + EPS)
            eln = setp.tile([128, 2 * NT], F32)
            nc.sync.dma_start(
                out=eln[:],
                in_=bass.AP(preds_d, 0, [[78, 128], [78 * 128, NT], [1, 2]]),
            )
            pr = setp.tile([128, 2 * NT], F32)
            sc.activation(pr[:], eln[:], Act.Sigmoid)
            l1 = setp.tile([128, 2 * NT], F32)
            sc.activation(l1[:], pr[:], Act.Ln, bias=ceps[:], scale=1.0)
            l2 = setp.tile([128, 2 * NT], F32)
            sc.activation(l2[:], pr[:], Act.Ln, bias=c1peps[:], scale=-1.0)
            q2 = setp.tile([128, 2 * NT], F32)
            sc.activation(q2[:], pr[:], Act.Square, bias=1.0, scale=-1.0)
            p2 = setp.tile([128, 2 * NT], F32)
            sc.activation(p2[:], pr[:], Act.Square)
            m1e = setp.tile([128, 2 * NT], F32)
            v.tensor_tensor(out=m1e[:], in0=l1[:], in1=q2[:], op=Alu.mult)
            m2e = setp.tile([128, 2 * NT], F32)
            v.tensor_tensor(out=m2e[:], in0=l2[:], in1=p2[:], op=Alu.mult)
            v.tensor_scalar(m2e[:], m2e[:], 0.75, None, op0=Alu.mult)
            ecols = setp.tile([128, 2 * NT], F32)
            v.scalar_tensor_tensor(
                ecols[:], m1e[:], -0.25, m2e[:], op0=Alu.mult, op1=Alu.add
            )
            e0c = setp.tile([128, NT], F32)
            v.tensor_copy(
                e0c[:], bass.AP(ecols.tensor, ecols.offset, [[2 * NT, 128], [2, NT]])
            )
            de_all = setp.tile([128, NT], F32)
            v.tensor_tensor(
                out=de_all[:],
                in0=bass.AP(ecols.tensor, ecols.offset + 1, [[2 * NT, 128], [2, NT]]),
                in1=e0c[:],
                op=Alu.subtract,
            )

            # ---------------- phase 1: per prior-tile ----------------
            # ends_all[:, i, 0] = 0; cols 1..32 = prefix at each target end
            ends_all = bigp.tile([128, NT * (T + 1)], F32)
            v.memset(ends_all[:], 0.0)
            P_all = bigp.tile([128, NTT], F32)

            for i in range(NT):
                px = predp.tile([128, S], F32, tag="px")
                nc.sync.dma_start(
                    out=px[:],
                    in_=bass.AP(preds_d, i * 128 * 78 + 6, [[78, 128], [1, S]]),
                )
                pfx = pfxp.tile([128, TS], F32, tag="pfx")
                pxv = bass.AP(px.tensor, px.offset, [list(px.ap[0]), [0, T], [1, S]])
                v._custom_dve(ABSDIFF_SCAN, out=pfx[:], in0=pxv, in1=txb[:])
                v.tensor_copy(
                    ends_all[:, i * (T + 1) + 1 : (i + 1) * (T + 1)],
                    bass.AP(pfx.tensor, pfx.offset + S - 1, [list(pfx.ap[0]), [S, T]]),
                )
                # P = px @ invm^T on PE (px transposed via PE first)
                p_pxT = psT.tile([S, 128], F32, tag="tr")
                te.transpose(p_pxT[:], px[:], ident[:])
                smat = sp.tile([S, 128], F32, tag="smat")
                sc.copy(smat[:], p_pxT[:])
                g = i // 4
                if i % 4 == 0:
                    pP = psP.tile([128, 128], F32, tag="pP", name=f"pP{g}")
                te.matmul(
                    pP[:, (i % 4) * T : (i % 4 + 1) * T],
                    smat[:],
                    invmC[:],
                    start=True,
                    stop=True,
                )
                if i % 4 == 3:
                    sc.copy(P_all[:, g * 128 : (g + 1) * 128], pP[:])

            # ---------------- batched [128, NT*T] passes ----------------
            dist_all = bigp.tile([128, NTT], F32)
            eh = bass.AP(
                ends_all.tensor, ends_all.offset + 1, [list(ends_all.ap[0]), [T + 1, NT], [1, T]]
            )
            el = bass.AP(
                ends_all.tensor, ends_all.offset, [list(ends_all.ap[0]), [T + 1, NT], [1, T]]
            )
            d3 = dist_all[:].rearrange("p (i t) -> p i t", t=T)
            v.tensor_tensor(out=d3, in0=eh, in1=el, op=Alu.subtract)
            v.tensor_tensor(out=dist_all[:], in0=dist_all[:], in1=P_all[:], op=Alu.subtract)
            v.tensor_tensor(out=d3, in0=d3, in1=_b3(invvlenb[:], "tile"), op=Alu.mult)

            # start-point distance
            sd_all = bigp.tile([128, NTT], F32)
            s3 = sd_all[:].rearrange("p (i t) -> p i t", t=T)
            dy_all = bigp.tile([128, NTT], F32)
            y3 = dy_all[:].rearrange("p (i t) -> p i t", t=T)
            v.tensor_tensor(out=s3, in0=_b3(sxtb[:], "tile"), in1=_b3(sxp[:], "t"), op=Alu.subtract)
            v.tensor_tensor(out=y3, in0=_b3(sytb[:], "tile"), in1=_b3(syp[:], "t"), op=Alu.subtract)
            sc.activation(sd_all[:], sd_all[:], Act.Square)
            sc.activation(dy_all[:], dy_all[:], Act.Square)
            v.tensor_tensor(out=sd_all[:], in0=sd_all[:], in1=dy_all[:], op=Alu.add)
            sc.activation(sd_all[:], sd_all[:], Act.Sqrt)

            # theta distance
            td_all = bigp.tile([128, NTT], F32)
            t3 = td_all[:].rearrange("p (i t) -> p i t", t=T)
            v.tensor_tensor(out=t3, in0=_b3(thtb[:], "tile"), in1=_b3(thp[:], "t"), op=Alu.subtract)
            sc.activation(td_all[:], td_all[:], Act.Abs)

            # focal classification cost (+BIG at invalid targets)
            cls_all = bigp.tile([128, NTT], F32)
            c3v = cls_all[:].rearrange("p (i t) -> p i t", t=T)
            v.tensor_tensor(out=c3v, in0=_b3(labb[:], "tile"), in1=_b3(de_all[:], "t"), op=Alu.mult)
            v.tensor_tensor(out=c3v, in0=c3v, in1=_b3(e0c[:], "t"), op=Alu.add)
            v.tensor_tensor(out=c3v, in0=c3v, in1=_b3(bigmask[:], "tile"), op=Alu.add)

            # ---------------- stats -> -1/max scalars ----------------
            mk3 = _b3(maskminb[:], "tile")

            def neg_inv_max(acc, name):
                mm = smp.tile([128, NTT], F32, tag="statscratch")
                v.tensor_tensor(
                    out=mm[:].rearrange("p (i t) -> p i t", t=T),
                    in0=acc[:].rearrange("p (i t) -> p i t", t=T),
                    in1=mk3,
                    op=Alu.min,
                )
                r = smp.tile([128, 1], F32, tag=name + "r")
                v.tensor_reduce(r[:], mm[:], axis=AX.X, op=Alu.max)
                ar = smp.tile([128, 1], F32, tag=name + "ar")
                gp.partition_all_reduce(
                    ar[:], r[:], channels=128, reduce_op=bass_isa.ReduceOp.max
                )
                v.tensor_scalar(ar[:], ar[:], 1.0e-6, None, op0=Alu.max)
                inv = smp.tile([128, 1], F32, tag=name + "inv")
                v.reciprocal(inv[:], ar[:])
                v.tensor_scalar(inv[:], inv[:], -1.0, None, op0=Alu.mult)
                return inv

            ninvd = neg_inv_max(dist_all, "d")
            ninvs = neg_inv_max(sd_all, "s")
            ninvt = neg_inv_max(td_all, "t")

            # ---------------- phase 2: negated cost ----------------
            a_ = bigp.tile([128, NTT], F32)
            v.tensor_scalar(a_[:], dist_all[:], ninvd[:], 1.01, op0=Alu.mult, op1=Alu.add)
            b_ = bigp.tile([128, NTT], F32)
            v.tensor_scalar(b_[:], sd_all[:], ninvs[:], 1.01, op0=Alu.mult, op1=Alu.add)
            c_ = bigp.tile([128, NTT], F32)
            v.tensor_scalar(c_[:], td_all[:], ninvt[:], 1.01, op0=Alu.mult, op1=Alu.add)
            v.tensor_tensor(out=a_[:], in0=a_[:], in1=b_[:], op=Alu.mult)
            v.tensor_tensor(out=a_[:], in0=a_[:], in1=c_[:], op=Alu.mult)
            # clamp (only reachable in invalid-target columns); keeps
            # 3*sq = 3e28 well below the 1e30 invalid-mask in cls_all
            v.tensor_scalar(a_[:], a_[:], -1.0e14, 1.0e14, op0=Alu.max, op1=Alu.min)
            sq = b_
            sc.activation(sq[:], a_[:], Act.Square)
            negcost = bigp.tile([128, NTT], F32)
            v.scalar_tensor_tensor(
                negcost[:], sq[:], 3.0, cls_all[:], op0=Alu.mult, op1=Alu.subtract
            )

            # ---------------- fold + Max8 top-4 ----------------
            def col_top8(mat_all, negate):
                """[T, 8] descending top-8 of (negate? -1 : 1)*columns."""
                folded = bigp.tile([128, 1024], F32, tag="folded" + str(negate))
                for g in range(8):
                    psg = psF.tile([T, 512], F32, tag="psg")
                    for j in range(4):
                        i = 4 * g + j
                        te.transpose(
                            psg[:, j * 128 : (j + 1) * 128],
                            mat_all[:, i * T : (i + 1) * T],
                            ident[:],
                        )
                    c = g // 2  # chunk of 1024 priors
                    sc.activation(
                        folded[c * T : (c + 1) * T, (g % 2) * 512 : (g % 2 + 1) * 512],
                        psg[:],
                        Act.Copy,
                        scale=-1.0 if negate else 1.0,
                    )
                cand = smp.tile([128, 8], F32, tag="cand8")
                v.max(out=cand[:], in_=folded[:])
                cg = smp.tile([T, 32], F32, tag="cg")
                for c in range(4):
                    v.tensor_copy(cg[:, 8 * c : 8 * (c + 1)], cand[c * T : (c + 1) * T, :])
                top8 = smp.tile([T, 8], F32, tag="top8")
                v.max(out=top8[:], in_=cg[:])
                return top8

            nd8 = col_top8(dist_all, True)  # top-8 of -distance
            dq = smp.tile([T, 4], F32, tag="dq")
            v.tensor_scalar(dq[:], nd8[:, 0:4], nvlenp[:], None, op0=Alu.mult)
            numq = smp.tile([T, 4], F32, tag="numq")
            v.tensor_scalar(numq[:], dq[:], -1.0, thirty[:], op0=Alu.mult, op1=Alu.add)
            denq = smp.tile([T, 4], F32, tag="denq")
            v.tensor_scalar(denq[:], dq[:], thirty_eps[:], None, op0=Alu.add)
            v.reciprocal(denq[:], denq[:])
            v.tensor_tensor(out=numq[:], in0=numq[:], in1=denq[:], op=Alu.mult)
            v.tensor_scalar(numq[:], numq[:], 0.0, None, op0=Alu.max)
            s4 = smp.tile([T, 1], F32, tag="s4")
            v.tensor_reduce(s4[:], numq[:], axis=AX.X, op=Alu.add)
            km1 = smp.tile([T, 1], F32, tag="km1")
            g3t = smp.tile([T, 1], F32, tag="g3t")
            v.tensor_scalar(km1[:], s4[:], 2.0, None, op0=Alu.is_ge)
            v.tensor_scalar(g3t[:], s4[:], 3.0, None, op0=Alu.is_ge)
            v.tensor_tensor(out=km1[:], in0=km1[:], in1=g3t[:], op=Alu.add)
            v.tensor_scalar(g3t[:], s4[:], 4.0, None, op0=Alu.is_ge)
            v.tensor_tensor(out=km1[:], in0=km1[:], in1=g3t[:], op=Alu.add)

            nc8 = col_top8(negcost, False)  # top-8 of -cost
            eqk = smp.tile([T, 4], F32, tag="eqk")
            v.tensor_scalar(eqk[:], iotaq[:, 0:4], km1[:], None, op0=Alu.is_equal)
            v.tensor_tensor(out=eqk[:], in0=eqk[:], in1=nc8[:, 0:4], op=Alu.mult)
            ck = smp.tile([T, 1], F32, tag="ck")
            v.tensor_reduce(ck[:], eqk[:], axis=AX.X, op=Alu.add)
            # invalid targets -> +BIG threshold
            v.tensor_tensor(out=ck[:], in0=ck[:], in1=maskcol[:], op=Alu.mult)
            mb = smp.tile([T, 1], F32, tag="mb")
            v.tensor_scalar(mb[:], maskcol[:], -BIG, BIG, op0=Alu.mult, op1=Alu.add)
            v.tensor_tensor(out=ck[:], in0=ck[:], in1=mb[:], op=Alu.add)
            ckr = smp.tile([1, T], F32, tag="ckr")
            nc.sync.dma_start(out=ckr[:], in_=ck[:])
            ckb = smp.tile([128, T], F32, tag="ckb")
            gp.partition_broadcast(ckb[:], ckr[:])

            # ---------------- selection / conflict / outputs ----------
            nv3 = negcost[:].rearrange("p (i t) -> p i t", t=T)
            mm = bigp.tile([128, NTT], F32)
            mm3 = mm[:].rearrange("p (i t) -> p i t", t=T)
            v.tensor_tensor(out=mm3, in0=nv3, in1=_b3(ckb[:], "tile"), op=Alu.is_ge)
            rs = smp.tile([128, NT], F32, tag="rs")
            v.tensor_reduce(rs[:], mm3, axis=AX.X, op=Alu.add)
            nmax = smp.tile([128, NT], F32, tag="nmax")
            v.tensor_reduce(nmax[:], nv3, axis=AX.X, op=Alu.max)
            oh = bigp.tile([128, NTT], F32)
            oh3 = oh[:].rearrange("p (i t) -> p i t", t=T)
            v.tensor_tensor(out=oh3, in0=nv3, in1=_b3(nmax[:], "t"), op=Alu.is_equal)
            conf = smp.tile([128, NT], F32, tag="conf")
            v.tensor_scalar(conf[:], rs[:], 1.0, None, op0=Alu.is_gt)
            # mm = mm + conf * (oh - mm)
            v.tensor_tensor(out=oh3, in0=oh3, in1=mm3, op=Alu.subtract)
            v.tensor_tensor(out=oh3, in0=oh3, in1=_b3(conf[:], "t"), op=Alu.mult)
            v.tensor_tensor(out=mm[:], in0=mm[:], in1=oh[:], op=Alu.add)
            asum = smp.tile([128, NT], F32, tag="asum")
            v.tensor_reduce(asum[:], mm3, axis=AX.X, op=Alu.max)
            t1 = oh  # reuse
            v.tensor_scalar(t1[:], mm[:], -1.0e9, 1.0e9, op0=Alu.mult, op1=Alu.add)
            t13 = t1[:].rearrange("p (i t) -> p i t", t=T)
            v.tensor_tensor(out=t13, in0=t13, in1=_b3(iota32t[:], "tile"), op=Alu.add)
            idxm = smp.tile([128, NT], F32, tag="idxm")
            v.tensor_reduce(idxm[:], t13, axis=AX.X, op=Alu.min)
            v.tensor_scalar(idxm[:], idxm[:], 1.0, None, op0=Alu.add)
            v.tensor_tensor(out=idxm[:], in0=idxm[:], in1=asum[:], op=Alu.mult)
            v.tensor_scalar(idxm[:], idxm[:], -1.0, None, op0=Alu.add)

            mi32 = smp.tile([128, NT], I32, tag="mi32")
            v.tensor_copy(mi32[:], idxm[:])
            au8 = smp.tile([128, NT], U8, tag="au8")
            v.tensor_copy(au8[:], asum[:])
            nc.sync.dma_start(out=bass.AP(mat_d, 0, [[1, 128], [128, NT]]), in_=mi32[:])
            nc.sync.dma_start(out=bass.AP(asn_d, 0, [[1, 128], [128, NT]]), in_=au8[:])

    nc.compile()
    return nc


_CACHE: dict[float, bass.Bass] = {}


def _get_nc(img_w: float) -> bass.Bass:
    if img_w not in _CACHE:
        _CACHE[img_w] = build(img_w)
    return _CACHE[img_w]


def _in_maps(preds, targets, masks, n_cores=8):
    B = preds.shape[0]
    ident, iotaq, iota32t = _host_constants()
    maps = []
    for c in range(n_cores):
        b = c % B
        maps.append(
            {
                "preds": np.ascontiguousarray(preds[b], np.float32),
                "targets": np.ascontiguousarray(targets[b], np.float32),
                "masks": np.ascontiguousarray(masks[b].reshape(1, T), np.int32),
                "ident": ident,
                "iotaq": iotaq,
                "iota32t": iota32t,
            }
        )
    return maps


def kernel(preds, targets, masks, img_w, img_h):
    del img_h
    B = preds.shape[0]
    nc = _get_nc(float(img_w))
    from concourse.bass_utils import run_bass_kernel_spmd

    res = run_bass_kernel_spmd(nc, _in_maps(preds, targets, masks), list(range(8)))
    outs = res.results[:B]
    assigned = np.stack([o["assigned"] for o in outs]).astype(bool)
    matched = np.stack([o["matched"] for o in outs]).astype(np.int32)
    return assigned, matched


# revision 19
# speedup vs baseline: 3.4115x; 1.7621x over previous
"""CLRNet SimOTA assignment kernel for Trainium2 (Bass/Tile).

Contract: kernel(**inputs) takes FULL inputs (preds [4,4096,78], targets
[4,32,78], masks [4,32], img_w, img_h) and returns (assigned [4,4096] bool,
matched [4,4096] int32) exactly like the reference.

Sharding: pure data parallel over the batch dim. B=4 images on 8 cores ->
each image runs on 2 cores (duplicated); outputs taken from cores 0..3.
Targets are host-compacted to the valid columns (padded to the batch-wide
max count Tc); the kernel gets the original indices as an input and is
correct for any input (rebuilds if a larger Tc is ever needed).

Key design (driven by measured TRN2 instruction costs):
  - D[n,t] = sum_s valid*|px[n,s]-tx[t,s]| via ONE custom-DVE scan op per
    prior tile: running prefix of |px_bcast - tx'| over the (t,s) free dim,
    written through a step-0 output AP so only each target-block's final
    prefix lands in SBUF (per-target sums = adjacent differences).
    tx' = tx*valid makes invalid points contribute |px| = px (px >= 0),
    removed by P[n,t] = px @ invalid_mask^T computed on the idle PE.
  - line IoU = (30*vcnt - D)/(30*vcnt + D + 1e-9), monotone decreasing in
    D per target, so top-4 iou priors = 4 smallest-distance priors ->
    Max8 on negated distances; cost top-4 likewise on negated cost with
    threshold selection (no indices anywhere).
  - All per-tile small ops batched into [128, NT*Tc] passes via step-0
    broadcast APs; outputs PE-transposed so the final DMA is contiguous.
  - No +/-inf: +/-1e30 sentinels.
"""

import os
import sys

import numpy as np

for _p in ("/opt/trn_rl_repo", "/root/.axon_site/_ro/trn_rl_repo"):
    if os.path.isdir(_p) and _p not in sys.path:
        sys.path.insert(0, _p)

import concourse.bacc as bacc  # noqa: E402
import concourse.bass as bass  # noqa: E402
import concourse.mybir as mybir  # noqa: E402
from concourse import bass_isa, dve_ops  # noqa: E402
from concourse.dve_spec import AluOp as DAlu  # noqa: E402
from concourse.dve_spec import Bin, Spec, Src0, Src1, lower, scan  # noqa: E402
from concourse.dve_uop import DveOpSpec  # noqa: E402
from concourse.tile import TileContext  # noqa: E402

F32 = mybir.dt.float32
I32 = mybir.dt.int32
U8 = mybir.dt.uint8
Alu = mybir.AluOpType
Act = mybir.ActivationFunctionType
AX = mybir.AxisListType

N = 4096
TFULL = 32
S = 72
NT = N // 128  # 32 prior tiles of 128
BIG = 1.0e30
EPS = 1.0e-12


def _register_absdiff_scan():
    """Custom DVE op: prefix-sum of |in0 - in1| along the free dim."""
    name = "ABSDIFF_SCAN_CLR"
    for op in dve_ops.OPS:
        if op.name == name:
            return op

    def _ref(in0, in1, s0, s1, imm2):
        a = in0.astype(np.float32).reshape(in0.shape[0], -1)
        b = in1.astype(np.float32).reshape(in1.shape[0], -1)
        d = np.abs(a - b)
        return np.cumsum(d, axis=-1).astype(np.float32)

    spec = Spec(
        body=scan(DAlu.ADD, Bin(DAlu.ABSOLUTE_DIFF, Src0, Src1)),
        reference=_ref,
    )
    shas = {}
    for ver in ("v3", "v4"):
        u = lower(spec, ver=ver)
        shas[ver] = DveOpSpec(name=name, opcode=0, uops=u, rd1_en=True).sha(ver)
    op = dve_ops.DveOp(name, spec, subdim=False, uops_sha=shas)
    dve_ops.OPS.append(op)
    dve_ops.CUSTOM_DVE_SPECS[name] = spec
    dve_ops._SUB_OPCODE_FOR_NAME[name] = (
        dve_ops._CUSTOM_DVE_ROW_BASE + len(dve_ops.OPS) - 1
    )
    return op


ABSDIFF_SCAN = _register_absdiff_scan()


def build(img_w: float, Tc: int) -> bass.Bass:
    T = Tc
    NTT = NT * T
    TS = T * S
    nc = bacc.Bacc("TRN2", target_bir_lowering=False, debug=False)

    preds_d = nc.dram_tensor("preds", [N, 78], F32, kind="ExternalInput")
    tgt_d = nc.dram_tensor("targets", [T, 78], F32, kind="ExternalInput")
    mask_d = nc.dram_tensor("masks", [1, T], I32, kind="ExternalInput")
    id_d = nc.dram_tensor("ident", [128, 128], F32, kind="ExternalInput")
    iq_d = nc.dram_tensor("iotaq", [T, 8], F32, kind="ExternalInput")
    ot_d = nc.dram_tensor("origt", [128, T], F32, kind="ExternalInput")
    asn_d = nc.dram_tensor("assigned", [N], U8, kind="ExternalOutput")
    mat_d = nc.dram_tensor("matched", [N], I32, kind="ExternalOutput")

    with TileContext(nc) as tc:
        with (
            tc.tile_pool(name="const", bufs=1) as constp,
            tc.tile_pool(name="setup", bufs=1) as setp,
            tc.tile_pool(name="big", bufs=1) as bigp,
            tc.tile_pool(name="ptile", bufs=4) as predp,
            tc.tile_pool(name="stile", bufs=2) as sp,
            tc.tile_pool(name="small", bufs=2) as smp,
            tc.tile_pool(name="psT", bufs=2, space="PSUM") as psT,
            tc.tile_pool(name="psP", bufs=2, space="PSUM") as psP,
            tc.tile_pool(name="psF", bufs=2, space="PSUM") as psF,
        ):
            v = nc.vector
            sc = nc.scalar
            gp = nc.gpsimd
            te = nc.tensor

            # ---------------- constants / inputs ----------------
            ident = constp.tile([128, 128], F32)
            nc.sync.dma_start(out=ident[:], in_=id_d.ap())
            iotaq = constp.tile([T, 8], F32)
            nc.sync.dma_start(out=iotaq[:], in_=iq_d.ap())
            origt = constp.tile([128, T], F32)
            nc.sync.dma_start(out=origt[:], in_=ot_d.ap())

            tgt = setp.tile([T, 78], F32)
            nc.sync.dma_start(out=tgt[:], in_=tgt_d.ap())
            maskrow_i = setp.tile([1, T], I32)
            nc.sync.dma_start(out=maskrow_i[:], in_=mask_d.ap())
            maskcol_i = setp.tile([T, 1], I32)
            gp.dma_start(out=maskcol_i[:], in_=bass.AP(mask_d, 0, [[1, T], [1, 1]]))
            maskrow = setp.tile([1, T], F32)
            v.tensor_copy(maskrow[:], maskrow_i[:])
            maskcol = setp.tile([T, 1], F32)
            v.tensor_copy(maskcol[:], maskcol_i[:])

            # per-(prior-tile) feature columns on the SWDGE queues
            def feat_cols(col, name):
                t = setp.tile([128, NT], F32, tag=name)
                gp.dma_start(
                    out=t[:],
                    in_=bass.AP(preds_d, col, [[78, 128], [78 * 128, NT], [1, 1]]),
                )
                return t

            sxp = feat_cols(2, "sxp")
            syp = feat_cols(3, "syp")
            thp = feat_cols(4, "thp")

            # ---------------- target-side prep (t-major [T, ...]) -------
            tx = tgt[:, 6:78]
            mge = setp.tile([T, S], F32)
            v.tensor_scalar(mge[:], tx, 0.0, None, op0=Alu.is_ge)
            mv = setp.tile([T, S], F32)
            v.tensor_scalar(mv[:], tx, float(img_w), None, op0=Alu.is_lt)
            v.tensor_tensor(out=mv[:], in0=mv[:], in1=mge[:], op=Alu.mult)

            vcnt = setp.tile([T, 1], F32)
            v.tensor_reduce(vcnt[:], mv[:], axis=AX.X, op=Alu.add)
            thirty = setp.tile([T, 1], F32)
            v.tensor_scalar(thirty[:], vcnt[:], 30.0, None, op0=Alu.mult)
            thirty_eps = setp.tile([T, 1], F32)
            v.tensor_scalar(
                thirty_eps[:], vcnt[:], 30.0, 1.0e-9, op0=Alu.mult, op1=Alu.add
            )
            vlenp = setp.tile([T, 1], F32)
            v.tensor_scalar(vlenp[:], vcnt[:], 1.0, 1.0e-6, op0=Alu.max, op1=Alu.add)
            invvlen = setp.tile([T, 1], F32)
            v.reciprocal(invvlen[:], vlenp[:])
            nvlenp = setp.tile([T, 1], F32)
            v.tensor_scalar(nvlenp[:], vlenp[:], -1.0, None, op0=Alu.mult)

            txn = setp.tile([T, S], F32)
            v.tensor_tensor(out=txn[:], in0=tx, in1=mv[:], op=Alu.mult)
            txrow = setp.tile([1, TS], F32)
            nc.sync.dma_start(out=txrow[:], in_=txn[:])
            txb = bigp.tile([128, TS], F32)
            gp.partition_broadcast(txb[:], txrow[:])

            invm = setp.tile([T, S], F32)
            v.tensor_scalar(invm[:], mv[:], -1.0, 1.0, op0=Alu.mult, op1=Alu.add)
            p_invT = psT.tile([S, T], F32, tag="tr")
            te.transpose(p_invT[:], invm[:], ident[0:T, 0:T])
            invmC = setp.tile([S, T], F32)
            sc.copy(invmC[:], p_invT[:])

            def tgt_row(col, name):
                t = setp.tile([1, T], F32, tag=name + "r")
                gp.dma_start(out=t[:], in_=bass.AP(tgt_d, col, [[1, 1], [78, T]]))
                return t

            def bcast128(src_row, name):
                t = setp.tile([128, T], F32, tag=name)
                gp.partition_broadcast(t[:], src_row)
                return t

            labb = bcast128(tgt_row(1, "labb")[:], "labb")
            sxtb = bcast128(tgt_row(2, "sxtb")[:], "sxtb")
            sytb = bcast128(tgt_row(3, "sytb")[:], "sytb")
            thtb = bcast128(tgt_row(4, "thtb")[:], "thtb")
            validb = bcast128(maskrow[:], "validb")
            maskminb = setp.tile([128, T], F32)
            v.tensor_scalar(
                maskminb[:], validb[:], 2.0 * BIG, -BIG, op0=Alu.mult, op1=Alu.add
            )
            bigmask = setp.tile([128, T], F32)
            v.tensor_scalar(
                bigmask[:], validb[:], -BIG, BIG, op0=Alu.mult, op1=Alu.add
            )
            invr = setp.tile([1, T], F32)
            nc.sync.dma_start(out=invr[:], in_=invvlen[:])
            invvlenb = bcast128(invr[:], "invvlenb")

            def b3(ap2d, axis):
                if axis == "tile":
                    return ap2d.unsqueeze(1).broadcast_to([128, NT, T])
                return ap2d.unsqueeze(2).broadcast_to([128, NT, T])

            # ---------------- focal-loss E columns ----------------
            ceps = setp.tile([128, 1], F32)
            v.memset(ceps[:], EPS)
            c1peps = setp.tile([128, 1], F32)
            v.memset(c1peps[:], 1.0 + EPS)
            eln = setp.tile([128, 2 * NT], F32)
            gp.dma_start(
                out=eln[:],
                in_=bass.AP(preds_d, 0, [[78, 128], [78 * 128, NT], [1, 2]]),
            )
            pr = setp.tile([128, 2 * NT], F32)
            sc.activation(pr[:], eln[:], Act.Sigmoid)
            l1 = setp.tile([128, 2 * NT], F32)
            sc.activation(l1[:], pr[:], Act.Ln, bias=ceps[:], scale=1.0)
            l2 = setp.tile([128, 2 * NT], F32)
            sc.activation(l2[:], pr[:], Act.Ln, bias=c1peps[:], scale=-1.0)
            q2 = setp.tile([128, 2 * NT], F32)
            sc.activation(q2[:], pr[:], Act.Square, bias=1.0, scale=-1.0)
            p2 = setp.tile([128, 2 * NT], F32)
            sc.activation(p2[:], pr[:], Act.Square)
            m1e = setp.tile([128, 2 * NT], F32)
            v.tensor_tensor(out=m1e[:], in0=l1[:], in1=q2[:], op=Alu.mult)
            m2e = setp.tile([128, 2 * NT], F32)
            v.tensor_tensor(out=m2e[:], in0=l2[:], in1=p2[:], op=Alu.mult)
            v.tensor_scalar(m2e[:], m2e[:], 0.75, None, op0=Alu.mult)
            ecols = setp.tile([128, 2 * NT], F32)
            v.scalar_tensor_tensor(
                ecols[:], m1e[:], -0.25, m2e[:], op0=Alu.mult, op1=Alu.add
            )
            e0c = setp.tile([128, NT], F32)
            v.tensor_copy(
                e0c[:], bass.AP(ecols.tensor, ecols.offset, [[2 * NT, 128], [2, NT]])
            )
            de_all = setp.tile([128, NT], F32)
            v.tensor_tensor(
                out=de_all[:],
                in0=bass.AP(ecols.tensor, ecols.offset + 1, [[2 * NT, 128], [2, NT]]),
                in1=e0c[:],
                op=Alu.subtract,
            )

            # ---------------- phase 1: scans + P matmuls ----------------
            ends_all = bigp.tile([128, NT * (T + 1)], F32)
            v.memset(ends_all[:], 0.0)
            P_all = bigp.tile([128, NTT], F32)
            GT = 4  # P-matmul tiles per psum group
            for i in range(NT):
                px = predp.tile([128, S], F32, tag="px")
                nc.sync.dma_start(
                    out=px[:],
                    in_=bass.AP(preds_d, i * 128 * 78 + 6, [[78, 128], [1, S]]),
                )
                pxv = bass.AP(px.tensor, px.offset, [list(px.ap[0]), [0, T], [1, S]])
                # scan writes only each 72-block's final prefix (step-0 inner)
                endv = bass.AP(
                    ends_all.tensor,
                    ends_all.offset + i * (T + 1) + 1,
                    [list(ends_all.ap[0]), [1, T], [0, S]],
                )
                v._custom_dve(ABSDIFF_SCAN, out=endv, in0=pxv, in1=txb[:])
                p_pxT = psT.tile([S, 128], F32, tag="tr")
                te.transpose(p_pxT[:], px[:], ident[:])
                smat = sp.tile([S, 128], F32, tag="smat")
                sc.copy(smat[:], p_pxT[:])
                g = i // GT
                if i % GT == 0:
                    pP = psP.tile([128, GT * T], F32, tag="pP", name=f"pP{g}")
                te.matmul(
                    pP[:, (i % GT) * T : (i % GT + 1) * T],
                    smat[:],
                    invmC[:],
                    start=True,
                    stop=True,
                )
                if i % GT == GT - 1:
                    sc.copy(P_all[:, g * GT * T : (g + 1) * GT * T], pP[:])

            # ---------------- batched [128, NT*T] passes ----------------
            dist_all = bigp.tile([128, NTT], F32)
            eh = bass.AP(
                ends_all.tensor,
                ends_all.offset + 1,
                [list(ends_all.ap[0]), [T + 1, NT], [1, T]],
            )
            el = bass.AP(
                ends_all.tensor,
                ends_all.offset,
                [list(ends_all.ap[0]), [T + 1, NT], [1, T]],
            )
            d3 = dist_all[:].rearrange("p (i t) -> p i t", t=T)
            v.tensor_tensor(out=d3, in0=eh, in1=el, op=Alu.subtract)
            v.tensor_tensor(
                out=dist_all[:], in0=dist_all[:], in1=P_all[:], op=Alu.subtract
            )
            v.tensor_tensor(out=d3, in0=d3, in1=b3(invvlenb[:], "tile"), op=Alu.mult)

            sd_all = bigp.tile([128, NTT], F32)
            s3 = sd_all[:].rearrange("p (i t) -> p i t", t=T)
            dy_all = bigp.tile([128, NTT], F32)
            y3 = dy_all[:].rearrange("p (i t) -> p i t", t=T)
            v.tensor_tensor(
                out=s3, in0=b3(sxtb[:], "tile"), in1=b3(sxp[:], "t"), op=Alu.subtract
            )
            v.tensor_tensor(
                out=y3, in0=b3(sytb[:], "tile"), in1=b3(syp[:], "t"), op=Alu.subtract
            )
            sc.activation(sd_all[:], sd_all[:], Act.Square)
            sc.activation(dy_all[:], dy_all[:], Act.Square)
            v.tensor_tensor(out=sd_all[:], in0=sd_all[:], in1=dy_all[:], op=Alu.add)
            sc.activation(sd_all[:], sd_all[:], Act.Sqrt)

            td_all = bigp.tile([128, NTT], F32)
            t3 = td_all[:].rearrange("p (i t) -> p i t", t=T)
            v.tensor_tensor(
                out=t3, in0=b3(thtb[:], "tile"), in1=b3(thp[:], "t"), op=Alu.subtract
            )
            sc.activation(td_all[:], td_all[:], Act.Abs)

            cls_all = bigp.tile([128, NTT], F32)
            c3v = cls_all[:].rearrange("p (i t) -> p i t", t=T)
            v.tensor_tensor(
                out=c3v, in0=b3(labb[:], "tile"), in1=b3(de_all[:], "t"), op=Alu.mult
            )
            v.tensor_tensor(out=c3v, in0=c3v, in1=b3(e0c[:], "t"), op=Alu.add)
            v.tensor_tensor(out=c3v, in0=c3v, in1=b3(bigmask[:], "tile"), op=Alu.add)

            # ---------------- stats -> -1/max scalars ----------------
            mk3 = b3(maskminb[:], "tile")

            def neg_inv_max(acc, name):
                mm = smp.tile([128, NTT], F32, tag="statscratch")
                v.tensor_tensor(
                    out=mm[:].rearrange("p (i t) -> p i t", t=T),
                    in0=acc[:].rearrange("p (i t) -> p i t", t=T),
                    in1=mk3,
                    op=Alu.min,
                )
                r = smp.tile([128, 1], F32, tag=name + "r")
                v.tensor_reduce(r[:], mm[:], axis=AX.X, op=Alu.max)
                ar = smp.tile([128, 1], F32, tag=name + "ar")
                gp.partition_all_reduce(
                    ar[:], r[:], channels=128, reduce_op=bass_isa.ReduceOp.max
                )
                v.tensor_scalar(ar[:], ar[:], 1.0e-6, None, op0=Alu.max)
                inv = smp.tile([128, 1], F32, tag=name + "inv")
                v.reciprocal(inv[:], ar[:])
                v.tensor_scalar(inv[:], inv[:], -1.0, None, op0=Alu.mult)
                return inv

            ninvd = neg_inv_max(dist_all, "d")
            ninvs = neg_inv_max(sd_all, "s")
            ninvt = neg_inv_max(td_all, "t")

            # ---------------- phase 2: negated cost ----------------
            a_ = bigp.tile([128, NTT], F32)
            v.tensor_scalar(a_[:], dist_all[:], ninvd[:], 1.01, op0=Alu.mult, op1=Alu.add)
            b_ = bigp.tile([128, NTT], F32)
            v.tensor_scalar(b_[:], sd_all[:], ninvs[:], 1.01, op0=Alu.mult, op1=Alu.add)
            c_ = bigp.tile([128, NTT], F32)
            v.tensor_scalar(c_[:], td_all[:], ninvt[:], 1.01, op0=Alu.mult, op1=Alu.add)
            v.tensor_tensor(out=a_[:], in0=a_[:], in1=b_[:], op=Alu.mult)
            v.tensor_tensor(out=a_[:], in0=a_[:], in1=c_[:], op=Alu.mult)
            v.tensor_scalar(a_[:], a_[:], -1.0e14, 1.0e14, op0=Alu.max, op1=Alu.min)
            sq = b_
            sc.activation(sq[:], a_[:], Act.Square)
            negcost = bigp.tile([128, NTT], F32)
            v.scalar_tensor_tensor(
                negcost[:], sq[:], 3.0, cls_all[:], op0=Alu.mult, op1=Alu.subtract
            )

            # ---------------- fold + Max8 top-4 ----------------
            def col_top8(mat_all, negate):
                folded = bigp.tile([128, 1024], F32, tag="folded" + str(negate))
                if T < 32:
                    v.memset(folded[:], -BIG)  # rows T..31 of each chunk unused
                for g in range(8):
                    psg = psF.tile([T, 512], F32, tag="psg")
                    for j in range(4):
                        i = 4 * g + j
                        te.transpose(
                            psg[:, j * 128 : (j + 1) * 128],
                            mat_all[:, i * T : (i + 1) * T],
                            ident[:],
                        )
                    c = g // 2
                    sc.activation(
                        folded[c * 32 : c * 32 + T, (g % 2) * 512 : (g % 2 + 1) * 512],
                        psg[:],
                        Act.Copy,
                        scale=-1.0 if negate else 1.0,
                    )
                cand = smp.tile([128, 8], F32, tag="cand8")
                v.max(out=cand[:], in_=folded[:])
                cg = smp.tile([T, 32], F32, tag="cg")
                for c in range(4):
                    v.tensor_copy(cg[:, 8 * c : 8 * (c + 1)], cand[c * 32 : c * 32 + T, :])
                top8 = smp.tile([T, 8], F32, tag="top8")
                v.max(out=top8[:], in_=cg[:])
                return top8

            nd8 = col_top8(dist_all, True)
            dq = smp.tile([T, 4], F32, tag="dq")
            v.tensor_scalar(dq[:], nd8[:, 0:4], nvlenp[:], None, op0=Alu.mult)
            numq = smp.tile([T, 4], F32, tag="numq")
            v.tensor_scalar(numq[:], dq[:], -1.0, thirty[:], op0=Alu.mult, op1=Alu.add)
            denq = smp.tile([T, 4], F32, tag="denq")
            v.tensor_scalar(denq[:], dq[:], thirty_eps[:], None, op0=Alu.add)
            v.reciprocal(denq[:], denq[:])
            v.tensor_tensor(out=numq[:], in0=numq[:], in1=denq[:], op=Alu.mult)
            v.tensor_scalar(numq[:], numq[:], 0.0, None, op0=Alu.max)
            s4 = smp.tile([T, 1], F32, tag="s4")
            v.tensor_reduce(s4[:], numq[:], axis=AX.X, op=Alu.add)
            km1 = smp.tile([T, 1], F32, tag="km1")
            g3t = smp.tile([T, 1], F32, tag="g3t")
            v.tensor_scalar(km1[:], s4[:], 2.0, None, op0=Alu.is_ge)
            v.tensor_scalar(g3t[:], s4[:], 3.0, None, op0=Alu.is_ge)
            v.tensor_tensor(out=km1[:], in0=km1[:], in1=g3t[:], op=Alu.add)
            v.tensor_scalar(g3t[:], s4[:], 4.0, None, op0=Alu.is_ge)
            v.tensor_tensor(out=km1[:], in0=km1[:], in1=g3t[:], op=Alu.add)

            nc8 = col_top8(negcost, False)
            eqk = smp.tile([T, 4], F32, tag="eqk")
            v.tensor_scalar(eqk[:], iotaq[:, 0:4], km1[:], None, op0=Alu.is_equal)
            v.tensor_tensor(out=eqk[:], in0=eqk[:], in1=nc8[:, 0:4], op=Alu.mult)
            ck = smp.tile([T, 1], F32, tag="ck")
            v.tensor_reduce(ck[:], eqk[:], axis=AX.X, op=Alu.add)
            v.tensor_tensor(out=ck[:], in0=ck[:], in1=maskcol[:], op=Alu.mult)
            mb = smp.tile([T, 1], F32, tag="mb")
            v.tensor_scalar(mb[:], maskcol[:], -BIG, BIG, op0=Alu.mult, op1=Alu.add)
            v.tensor_tensor(out=ck[:], in0=ck[:], in1=mb[:], op=Alu.add)
            ckr = smp.tile([1, T], F32, tag="ckr")
            nc.sync.dma_start(out=ckr[:], in_=ck[:])
            ckb = smp.tile([128, T], F32, tag="ckb")
            gp.partition_broadcast(ckb[:], ckr[:])

            # ---------------- selection / conflict / outputs ----------
            nv3 = negcost[:].rearrange("p (i t) -> p i t", t=T)
            mm = bigp.tile([128, NTT], F32)
            mm3 = mm[:].rearrange("p (i t) -> p i t", t=T)
            v.tensor_tensor(out=mm3, in0=nv3, in1=b3(ckb[:], "tile"), op=Alu.is_ge)
            rs = smp.tile([128, NT], F32, tag="rs")
            v.tensor_reduce(rs[:], mm3, axis=AX.X, op=Alu.add)
            nmax = smp.tile([128, NT], F32, tag="nmax")
            v.tensor_reduce(nmax[:], nv3, axis=AX.X, op=Alu.max)
            oh = bigp.tile([128, NTT], F32)
            oh3 = oh[:].rearrange("p (i t) -> p i t", t=T)
            v.tensor_tensor(out=oh3, in0=nv3, in1=b3(nmax[:], "t"), op=Alu.is_equal)
            conf = smp.tile([128, NT], F32, tag="conf")
            v.tensor_scalar(conf[:], rs[:], 1.0, None, op0=Alu.is_gt)
            v.tensor_tensor(out=oh3, in0=oh3, in1=mm3, op=Alu.subtract)
            v.tensor_tensor(out=oh3, in0=oh3, in1=b3(conf[:], "t"), op=Alu.mult)
            v.tensor_tensor(out=mm[:], in0=mm[:], in1=oh[:], op=Alu.add)
            asum = smp.tile([128, NT], F32, tag="asum")
            v.tensor_reduce(asum[:], mm3, axis=AX.X, op=Alu.max)
            t1 = oh
            v.tensor_scalar(t1[:], mm[:], -1.0e9, 1.0e9, op0=Alu.mult, op1=Alu.add)
            t13 = t1[:].rearrange("p (i t) -> p i t", t=T)
            v.tensor_tensor(out=t13, in0=t13, in1=b3(origt[:], "tile"), op=Alu.add)
            idxm = smp.tile([128, NT], F32, tag="idxm")
            v.tensor_reduce(idxm[:], t13, axis=AX.X, op=Alu.min)
            # matched = assigned * (idx+1) - 1 ; then transpose for output
            v.tensor_scalar(idxm[:], idxm[:], 1.0, None, op0=Alu.add)
            v.tensor_tensor(out=idxm[:], in0=idxm[:], in1=asum[:], op=Alu.mult)
            v.tensor_scalar(idxm[:], idxm[:], -1.0, None, op0=Alu.add)

            p_mT = psF.tile([NT, 128], F32, tag="psg", name="p_mT")
            te.transpose(p_mT[:], idxm[:], ident[:])
            mTf = smp.tile([NT, 128], F32, tag="mTf")
            sc.copy(mTf[:], p_mT[:])
            mT32 = smp.tile([NT, 128], I32, tag="mT32")
            v.tensor_copy(mT32[:], mTf[:])
            p_aT = psF.tile([NT, 128], F32, tag="psg", name="p_aT")
            te.transpose(p_aT[:], asum[:], ident[:])
            aTf = smp.tile([NT, 128], F32, tag="aTf")
            sc.copy(aTf[:], p_aT[:])
            aT8 = smp.tile([NT, 128], U8, tag="aT8")
            v.tensor_copy(aT8[:], aTf[:])
            nc.sync.dma_start(out=bass.AP(mat_d, 0, [[128, NT], [1, 128]]), in_=mT32[:])
            nc.sync.dma_start(out=bass.AP(asn_d, 0, [[128, NT], [1, 128]]), in_=aT8[:])

    nc.compile()
    return nc


_CACHE: dict[tuple, bass.Bass] = {}


def _get_nc(img_w: float, Tc: int) -> bass.Bass:
    key = (img_w, Tc)
    if key not in _CACHE:
        _CACHE[key] = build(img_w, Tc)
    return _CACHE[key]


def _compact(targets, masks):
    """Keep only valid target columns, padded to the batch max count."""
    B = targets.shape[0]
    counts = [int(masks[b].sum()) for b in range(B)]
    Tc = max(1, max(counts))
    ct = np.zeros((B, Tc, 78), np.float32)
    cm = np.zeros((B, Tc), np.int32)
    ot = np.zeros((B, Tc), np.float32)
    for b in range(B):
        idx = np.nonzero(masks[b])[0]
        k = len(idx)
        if k:
            ct[b, :k] = targets[b, idx]
            ot[b, :k] = idx.astype(np.float32)
            cm[b, :k] = 1
    return Tc, ct, cm, ot


def _in_maps(preds, targets, masks, n_cores=8):
    B = preds.shape[0]
    Tc, ct, cm, ot = _compact(np.asarray(targets), np.asarray(masks))
    ident = np.eye(128, dtype=np.float32)
    iotaq = np.tile(np.arange(8, dtype=np.float32)[None, :], (Tc, 1))
    maps = []
    for c in range(n_cores):
        b = c % B
        maps.append(
            {
                "preds": np.ascontiguousarray(preds[b], np.float32),
                "targets": np.ascontiguousarray(ct[b], np.float32),
                "masks": np.ascontiguousarray(cm[b].reshape(1, Tc), np.int32),
                "ident": ident,
                "iotaq": iotaq,
                "origt": np.ascontiguousarray(
                    np.tile(ot[b][None, :], (128, 1)), np.float32
                ),
            }
        )
    return Tc, maps


def kernel(preds, targets, masks, img_w, img_h):
    del img_h
    B = preds.shape[0]
    Tc, maps = _in_maps(preds, targets, masks)
    nc = _get_nc(float(img_w), Tc)
    from concourse.bass_utils import run_bass_kernel_spmd

    res = run_bass_kernel_spmd(nc, maps, list(range(8)))
    outs = res.results[:B]
    assigned = np.stack([o["assigned"] for o in outs]).astype(bool)
    matched = np.stack([o["matched"] for o in outs]).astype(np.int32)
    return assigned, matched
